# revision 1
# baseline (speedup 1.0000x reference)
"""Trainium2 Bass kernel for CellFoundation self-attention (B=4, S=1024, HID=1024, H=16, DH=64).

Sharding: 8 cores as 4 (batch) x 2 (head-group) grid. Each core handles one
batch and 8 heads (512 of the 1024 QKV output dims).

Per-core device pipeline (everything transposed, tokens in the free dim):
  - Q^T/K^T projection: lhsT=W-chunk [hid,od], rhs=X^T-chunk [hid,tok] -> psum [od,tok]
  - RoPE fused: tcos=(psum+b)*cosT, ysin=(psum+b)*sinT' (DVE scalar_tensor_tensor),
    rot = R @ ysin (PE permutation matmul reusing the proj psum slot; signs are
    host-baked into sinT'), qrope = tcos + rot. Valid because the RoPE freqs
    duplicate halves, so the rotate-half sin factor equals the source-index sin.
  - V projection in natural layout: lhsT=X^T-chunk [hid,tok], rhs=Wv [hid,od]
    -> psum [tok,od], evicted into per-head 65-column slots with a ones column.
  - scores^T[k,q] = K^T.T @ Q^T per head; two heads in PE row groups (K=64,
    base partitions 0/64). For FP8_SCORE_PAIRS the RoPE outputs are written
    as fp8 e4m3 and the matmul runs in DoubleRow perf mode (0.5 cyc/row) with
    BOTH operands broadcast (stride-0) across the two DR slots -> computes
    2x(K.T@Q), absorbed into the exp scale (0.0625 vs 0.125). Halves score
    matmul PE time for ~+0.9e-2 quadrature rel-err per operand pair.
  - exp fused on ACT: E = exp(scale*scores + mask_bias) (psum -> sbuf bf16).
    No max-subtraction needed: |scores/8| < ~5; masked keys get bias -60.
  - ctx^T unnormalized: psum [65,q] += V_hat[k,0:65].T @ E[k,q] over k-chunks;
    row 64 = softmax denominator (ones column).
  - evict ctx psum early to SBUF, then recip (DVE) + partition_broadcast
    (GpSimd) + multiply (DVE) off the critical path.
Host: shard/transpose/pack inputs (numpy), gather + transpose outputs.
"""

import sys

if "/opt/trn_rl_repo" not in sys.path:
    sys.path.insert(0, "/opt/trn_rl_repo")

from contextlib import ExitStack

import ml_dtypes
import numpy as np

import concourse.bass as bass  # noqa: F401
import concourse.tile as tile
from concourse import bacc, mybir
from concourse.bass_utils import run_bass_kernel_spmd

F32 = mybir.dt.float32
BF16 = mybir.dt.bfloat16
AF = mybir.ActivationFunctionType
MUL = mybir.AluOpType.mult
ADD = mybir.AluOpType.add

B, S, HID, H, DH = 4, 1024, 1024, 16, 64
P = 128
NCORES = 8
HG = 2
HL = H // HG        # 8 local heads
OD = HL * DH        # 512
KC = HID // P       # 8
NT = S // P         # 8
QCH = 512
NQC = S // QCH      # 2
NPAIR = HL // 2     # 4
MASK_NEG = -60.0
FP8_QK = False
W_SCALE = 32.0
F8 = mybir.dt.float8e4
# Pairs (of 2 heads) whose scores matmul runs in fp8 DoubleRow: Q/K rope
# outputs quantized to e4m3, both operands broadcast across the two DR slots
# (score x2, absorbed into the exp scale). Each fp8 pair saves ~1.7us PE at
# ~+0.9e-2 quadrature rel-err (f=1.0 -> ~1.8e-2 total vs 2e-2 gate).
FP8_SCORE_PAIRS = (0, 1, 2, 3)


def _build_nc(debug: bool = False):
    nc = bacc.Bacc("TRN2", target_bir_lowering=False, debug=debug)

    d_xt = nc.dram_tensor("xt", [HID, S], BF16, kind="ExternalInput")
    WDT = F8 if FP8_QK else BF16
    if FP8_QK:
        d_xt8 = nc.dram_tensor("xt8", [HID, S], F8, kind="ExternalInput")
    d_wq = nc.dram_tensor("wq", [HID, OD], WDT, kind="ExternalInput")
    d_wk = nc.dram_tensor("wk", [HID, OD], WDT, kind="ExternalInput")
    d_wv = nc.dram_tensor("wv", [HID, OD], BF16, kind="ExternalInput")
    d_cos = nc.dram_tensor("cos2", [P, S], BF16, kind="ExternalInput")
    d_sin = nc.dram_tensor("sin2", [P, S], BF16, kind="ExternalInput")  # sign-baked
    d_R = nc.dram_tensor("rmat", [P, P], BF16, kind="ExternalInput")
    d_bq = nc.dram_tensor("bq", [P, OD // P], F32, kind="ExternalInput")
    d_bk = nc.dram_tensor("bk", [P, OD // P], F32, kind="ExternalInput")
    d_bvb = nc.dram_tensor("bvb", [1, OD], F32, kind="ExternalInput")
    d_mask = nc.dram_tensor("maskt", [P, NT], F32, kind="ExternalInput")
    d_out = nc.dram_tensor("out", [OD, S], F32, kind="ExternalOutput")

    with tile.TileContext(nc) as tc, ExitStack() as ctx:
        const = ctx.enter_context(tc.tile_pool(name="const", bufs=1))
        qkp = ctx.enter_context(tc.tile_pool(name="qkp", bufs=4))
        rt = ctx.enter_context(tc.tile_pool(name="rt", bufs=8))
        ep = ctx.enter_context(tc.tile_pool(name="ep", bufs=10))
        npool = ctx.enter_context(tc.tile_pool(name="npool", bufs=6))
        pp = ctx.enter_context(tc.tile_pool(name="pp", bufs=2, space="PSUM"))
        sp = ctx.enter_context(tc.tile_pool(name="sp", bufs=2, space="PSUM"))
        cp = ctx.enter_context(tc.tile_pool(name="cp", bufs=2, space="PSUM"))

        # ---- constants / inputs to SBUF ----
        t_cos = const.tile([P, S], BF16)
        t_sin = const.tile([P, S], BF16)
        t_bq = const.tile([P, OD // P], F32)
        t_bk = const.tile([P, OD // P], F32)
        t_bvb = const.tile([P, OD], F32)
        t_mask = const.tile([P, NT], F32)
        t_R = const.tile([P, P], BF16)
        nc.gpsimd.dma_start(t_mask[:], d_mask[:])
        nc.gpsimd.dma_start(t_cos[:], d_cos[:])
        nc.gpsimd.dma_start(t_sin[:], d_sin[:])
        nc.gpsimd.dma_start(t_bq[:], d_bq[:])
        nc.gpsimd.dma_start(t_bk[:], d_bk[:])
        nc.gpsimd.dma_start(t_bvb[:], d_bvb[:].to_broadcast([P, OD]))
        nc.gpsimd.dma_start(t_R[:], d_R[:])

        t_xt = [const.tile([P, S], BF16, tag=f"xt{kk}", name=f"xt{kk}") for kk in range(KC)]
        if FP8_QK:
            t_wq = const.tile([P, KC // 2, 2, OD], F8)
            t_wk = const.tile([P, KC // 2, 2, OD], F8)
            t_xt8 = [
                const.tile([P, 2, S], F8, tag=f"xt8_{c}", name=f"xt8_{c}")
                for c in range(KC // 2)
            ]
        else:
            t_wq = [const.tile([P, OD], BF16, tag=f"twq{kk}", name=f"twq{kk}") for kk in range(KC)]
            t_wk = [const.tile([P, OD], BF16, tag=f"twk{kk}", name=f"twk{kk}") for kk in range(KC)]
        t_wv = [const.tile([P, OD], BF16, tag=f"wv{kk}", name=f"wv{kk}") for kk in range(KC)]
        xt_r = d_xt[:].rearrange("(c p) t -> p c t", p=P)
        if FP8_QK:
            wq_r = d_wq[:].rearrange("(c i p) o -> p c i o", p=P, i=2)
            wk_r = d_wk[:].rearrange("(c i p) o -> p c i o", p=P, i=2)
            xt8_r = d_xt8[:].rearrange("(c i p) t -> p c i t", p=P, i=2)
        else:
            wq_r = d_wq[:].rearrange("(c p) o -> p c o", p=P)
            wk_r = d_wk[:].rearrange("(c p) o -> p c o", p=P)
        wv_r = d_wv[:].rearrange("(c p) o -> p c o", p=P)
        for kk in range(KC):
            nc.sync.dma_start(t_xt[kk][:], xt_r[:, kk, :])
            nc.sync.dma_start(t_wv[kk][:], wv_r[:, kk, :])
        if FP8_QK:
            nc.sync.dma_start(t_wq[:], wq_r[:])
            nc.sync.dma_start(t_wk[:], wk_r[:])
            for c in range(KC // 2):
                nc.sync.dma_start(t_xt8[c][:], xt8_r[:, c, :, :])
        else:
            for kk in range(KC):
                nc.sync.dma_start(t_wq[kk][:], wq_r[:, kk, :])
                nc.sync.dma_start(t_wk[kk][:], wk_r[:, kk, :])

        # ---- PE HAM warmup: ~3.4us of dummy matmuls during the input-DMA
        # wait releases the clock gate (1.2 -> 2.4 GHz) before real work.
        # memset needs no DMA, so these start immediately. Model-invisible
        # (cost model ramps too), real-HW win.
        t_wu = rt.tile([P, OD], BF16, tag="wu", bufs=1)
        nc.vector.memset(t_wu[:], 0.0)
        p_wu = pp.tile([P, OD], F32, tag="pp")
        for _ in range(14):
            nc.tensor.matmul(p_wu[0:P, :], t_wu[:, 0:P], t_wu[:], start=True, stop=True)

        # ---- QK projection + RoPE helpers (defined early; used in V loop) ----
        def qk_proj_chunk(hp, which, dest, t):
            t_w = t_wq if which == "q" else t_wk
            t_b = t_bq if which == "q" else t_bk
            tsl = slice(t * QCH, (t + 1) * QCH)
            p_q = pp.tile([P, QCH], F32, tag="pp")
            if FP8_QK:
                for c in range(KC // 2):
                    nc.tensor.matmul(
                        p_q,
                        t_w[:, c, :, hp * P : (hp + 1) * P],
                        t_xt8[c][:, :, tsl],
                        start=(c == 0),
                        stop=(c == KC // 2 - 1),
                        perf_mode=mybir.MatmulPerfMode.DoubleRow,
                    )
            else:
                for kk in range(KC):
                    nc.tensor.matmul(
                        p_q,
                        t_w[kk][:, hp * P : (hp + 1) * P],
                        t_xt[kk][:, tsl],
                        start=(kk == 0),
                        stop=(kk == KC - 1),
                    )
            t_tc = rt.tile([P, QCH], BF16, tag="tc")
            t_ys = rt.tile([P, QCH], BF16, tag="ys")
            nc.vector.scalar_tensor_tensor(
                out=t_ys, in0=p_q, scalar=t_b[:, hp : hp + 1],
                in1=t_sin[:, tsl], op0=ADD, op1=MUL,
            )
            nc.vector.scalar_tensor_tensor(
                out=t_tc, in0=p_q, scalar=t_b[:, hp : hp + 1],
                in1=t_cos[:, tsl], op0=ADD, op1=MUL,
            )
            # rotate-half via PE permutation matmul (signs live in t_sin);
            # reuse p_q: WAR on the stt reads serializes correctly, saves a slot
            nc.tensor.matmul(p_q, t_R[:], t_ys, start=True, stop=True)
            nc.vector.tensor_add(dest[:], t_tc, p_q)

        def qk_chunks(hp, t_qr, t_kr):
            return [
                lambda w=w, d=d, t=t: qk_proj_chunk(hp, w, d[t], t)
                for w, d in (("q", t_qr), ("k", t_kr))
                for t in range(NQC)
            ]

        def qk_tiles(hp):
            dt_ = F8 if hp in FP8_SCORE_PAIRS else BF16
            qr = [qkp.tile([P, QCH], dt_, tag=f"qr{t}", name=f"qr{t}_{hp}") for t in range(NQC)]
            kr = [qkp.tile([P, QCH], dt_, tag=f"kr{t}", name=f"kr{t}_{hp}") for t in range(NQC)]
            return qr, kr

        # preload the exp activation table while input DMAs stream
        t_warm = rt.tile([1, 4], F32, tag="warm")
        nc.scalar.activation(t_warm, t_mask[0:1, 0:4], AF.Exp)

        # ---- V projection into per-head 65-col slots (ones col = denominator) ----
        t_v = [const.tile([P, HL * 65], BF16, tag=f"v{vt}", name=f"vsb{vt}") for vt in range(NT)]
        t_qr0, t_kr0 = qk_tiles(0)
        _c0 = qk_chunks(0, t_qr0, t_kr0)  # [q0, q1, k0, k1]
        # first scores need q0+k0; k1 not until kt4; q1 not until qc1
        pair0_chunks = [_c0[0], _c0[2], _c0[1], _c0[3]]
        for vt in range(NT):
            # alternate between the two psum pools so chains overlap
            pool_tag = (pp, "pp") if vt % 2 == 0 else (sp, "sp")
            p_v = pool_tag[0].tile([P, OD], F32, tag=pool_tag[1])
            for kk in range(KC):
                nc.tensor.matmul(
                    p_v,
                    t_xt[kk][:, vt * P : (vt + 1) * P],
                    t_wv[kk][:],
                    start=(kk == 0),
                    stop=(kk == KC - 1),
                )
            nc.vector.tensor_add(
                t_v[vt][:].rearrange("p (h c) -> p h c", h=HL, c=65)[:, :, 0:64],
                p_v[:].rearrange("p (h c) -> p h c", h=HL, c=64),
                t_bvb[:].rearrange("p (h c) -> p h c", h=HL, c=64),
            )
            nc.vector.memset(
                t_v[vt][:].rearrange("p (h c) -> p h c", h=HL, c=65)[:, :, 64:65], 1.0
            )
            if vt in (4, 5, 6, 7) and pair0_chunks:
                pair0_chunks.pop(0)()

        qr_cur, kr_cur = t_qr0, t_kr0

        # ---- attention per pair; next pair's proj chunks interleaved ----
        self_chunks = []
        for hp in range(NPAIR):
            nxt = list(self_chunks)
            self_chunks = []
            qr_nxt = kr_nxt = None
            if hp + 1 < NPAIR:
                qr_nxt, kr_nxt = qk_tiles(hp + 1)
                chunks = qk_chunks(hp + 1, qr_nxt, kr_nxt)  # [q0, q1, k0, k1]
                if hp + 1 == NPAIR - 1:
                    # last pair: only q0/k0 ride in this pair's attention; its
                    # k1/q1 fill the last pair's own ACT-bound stretch
                    nxt += [chunks[0], chunks[2]]
                    self_chunks = [chunks[3], chunks[1]]
                else:
                    # q0/k0 first: they gate the next pair's first scores
                    nxt += [chunks[0], chunks[2], chunks[1], chunks[3]]
            h0, h1 = 2 * hp, 2 * hp + 1
            for qc in range(NQC):
                qsl = slice(qc * QCH, (qc + 1) * QCH)
                p_c0 = cp.tile([65, QCH], F32, tag="cp")
                p_c1 = cp.tile([65, QCH], F32, tag="cp")
                es = []
                fp8_pair = hp in FP8_SCORE_PAIRS
                for kt in range(NT):
                    p_s = sp.tile([P, 2 * QCH], F32, tag="sp")
                    krc = kr_cur[kt // 4]
                    kcol = (kt % 4) * P
                    for half, b0 in ((0, 0), (1, 64)):
                        osl = slice(half * QCH, (half + 1) * QCH)
                        if fp8_pair:
                            # both operands broadcast across the two DoubleRow
                            # slots -> 2x the score, absorbed into exp scale
                            nc.tensor.matmul(
                                p_s[:, osl],
                                krc[b0 : b0 + 64, kcol : kcol + P]
                                .unsqueeze(1).to_broadcast([64, 2, P]),
                                qr_cur[qc][b0 : b0 + 64, :]
                                .unsqueeze(1).to_broadcast([64, 2, QCH]),
                                start=True, stop=True,
                                perf_mode=mybir.MatmulPerfMode.DoubleRow,
                            )
                        else:
                            nc.tensor.matmul(
                                p_s[:, osl],
                                krc[b0 : b0 + 64, kcol : kcol + P],
                                qr_cur[qc][b0 : b0 + 64, :],
                                start=True, stop=True,
                            )
                    t_e = ep.tile([P, 2 * QCH], BF16, tag="e")
                    nc.scalar.activation(
                        t_e, p_s, AF.Exp, bias=t_mask[:, kt : kt + 1],
                        scale=0.0625 if fp8_pair else 0.125,
                    )
                    es.append(t_e)
                    if kt in (0, 2, 4, 6) and nxt and (kt in (0, 4) or len(nxt) > (1 - kt // 4)):
                        nxt.pop(0)()
                if hp == NPAIR - 1 and qc == NQC - 1:
                    # last unit: finish h0's accumulation first so its
                    # normalize chain overlaps h1's PV matmuls
                    for kt in range(NT):
                        nc.tensor.matmul(
                            p_c0,
                            t_v[kt][:, h0 * 65 : h0 * 65 + 65],
                            es[kt][:, 0:QCH],
                            start=(kt == 0), stop=(kt == NT - 1),
                        )
                    for kt in range(NT):
                        nc.tensor.matmul(
                            p_c1,
                            t_v[kt][:, h1 * 65 : h1 * 65 + 65],
                            es[kt][:, QCH : 2 * QCH],
                            start=(kt == 0), stop=(kt == NT - 1),
                        )
                else:
                    for kt in range(NT):
                        nc.tensor.matmul(
                            p_c0,
                            t_v[kt][:, h0 * 65 : h0 * 65 + 65],
                            es[kt][:, 0:QCH],
                            start=(kt == 0), stop=(kt == NT - 1),
                        )
                        nc.tensor.matmul(
                            p_c1,
                            t_v[kt][:, h1 * 65 : h1 * 65 + 65],
                            es[kt][:, QCH : 2 * QCH],
                            start=(kt == 0), stop=(kt == NT - 1),
                        )
                last_unit = hp == NPAIR - 1 and qc == NQC - 1
                for h, p_c in ((h0, p_c0), (h1, p_c1)):
                    t_rr = npool.tile([1, QCH], F32, tag="rr")
                    if last_unit:
                        # tail: skip the eviction copy (nothing reuses the cp
                        # slot) and shorten the dependency chain
                        src_v = p_c
                        nc.vector.reciprocal(t_rr, p_c[64:65, :])
                    else:
                        # steady state: evict psum early so the cp slot
                        # recycles ahead of the next qc's PV pass
                        t_cu = npool.tile([65, QCH], F32, tag="cu")
                        nc.vector.tensor_copy(t_cu, p_c)
                        src_v = t_cu
                        nc.vector.reciprocal(t_rr, t_cu[64:65, :])
                    t_rb = npool.tile([64, QCH], F32, tag="rb")
                    nc.gpsimd.partition_broadcast(t_rb, t_rr)
                    t_ctx = npool.tile([64, QCH], F32, tag="ctx")
                    nc.vector.tensor_mul(t_ctx, src_v[0:64, :], t_rb)
                    nc.sync.dma_start(d_out[h * 64 : (h + 1) * 64, qsl], t_ctx)
            if hp + 1 < NPAIR:
                for fn in nxt:
                    fn()
                qr_cur, kr_cur = qr_nxt, kr_nxt

    nc.compile()
    return nc


_NC_CACHE = {}


def _get_nc(debug: bool = False):
    key = bool(debug)
    if key not in _NC_CACHE:
        _NC_CACHE[key] = _build_nc(debug)
    return _NC_CACHE[key]


def _prep_inputs(hidden_states, attention_mask, freqs, Wq, bq, Wk, bk, Wv, bv):
    # coerce to numpy: setup_inputs() returns jax arrays, and host-side prep
    # must not dispatch onto the accelerator backend
    hidden_states = np.asarray(hidden_states)
    attention_mask = np.asarray(attention_mask)
    freqs = np.asarray(freqs)
    Wq, bq = np.asarray(Wq), np.asarray(bq)
    Wk, bk = np.asarray(Wk), np.asarray(bk)
    Wv, bv = np.asarray(Wv), np.asarray(bv)
    bf = ml_dtypes.bfloat16

    cosf = np.cos(freqs.astype(np.float64)).astype(np.float32)  # [S, 64]
    sinf = np.sin(freqs.astype(np.float64)).astype(np.float32)
    qk_scale = (1.0 / W_SCALE) if FP8_QK else 1.0
    cos2 = (np.tile(cosf.T, (2, 1)) * qk_scale).astype(bf)  # [128, S]
    # sign-baked sin for the DMA rotate-half: within each 64 block,
    # rows 0:32 keep +sin (their values land on rows 32:64 with + sign),
    # rows 32:64 get -sin (their values land on rows 0:32 with - sign).
    sgn = np.ones((64, 1), np.float32)
    sgn[32:] = -1.0
    sin2 = (np.tile(sinf.T * sgn, (2, 1)) * qk_scale).astype(bf)  # [128, S]

    rmat = np.zeros((P, P), np.float32)
    for blk in (0, 64):
        for i in range(32):
            rmat[blk + i + 32, blk + i] = 1.0      # dest i<32 <- src i+32
            rmat[blk + i, blk + i + 32] = 1.0      # dest i>=32 <- src i-32
    rmat = rmat.astype(bf)

    xts = [np.ascontiguousarray(hidden_states[b].T).astype(bf) for b in range(B)]
    if FP8_QK:
        f8 = mybir.dt.np(F8)
        xt8s = [np.ascontiguousarray(hidden_states[b].T).astype(f8) for b in range(B)]
    masks = []
    for b in range(B):
        m = np.where(attention_mask[b, 0, 0, :] < -1e-5, MASK_NEG, 0.0).astype(
            np.float32
        )
        masks.append(np.ascontiguousarray(m.reshape(NT, P).T))

    wqs, wks, wvs, bqs, bks, bvs = [], [], [], [], [], []
    for g in range(HG):
        osl = slice(g * OD, (g + 1) * OD)
        if FP8_QK:
            f8 = mybir.dt.np(F8)
            wqs.append(np.ascontiguousarray(Wq[:, osl] * W_SCALE).astype(f8))
            wks.append(np.ascontiguousarray(Wk[:, osl] * W_SCALE).astype(f8))
        else:
            wqs.append(np.ascontiguousarray(Wq[:, osl]).astype(bf))
            wks.append(np.ascontiguousarray(Wk[:, osl]).astype(bf))
        wvs.append(np.ascontiguousarray(Wv[:, osl]).astype(bf))
        wsc = W_SCALE if FP8_QK else 1.0
        bqs.append(np.ascontiguousarray((bq[osl] * wsc).reshape(OD // P, P).T.astype(np.float32)))
        bks.append(np.ascontiguousarray((bk[osl] * wsc).reshape(OD // P, P).T.astype(np.float32)))
        bvs.append(bv[osl].reshape(1, OD).astype(np.float32))

    in_maps = []
    for c in range(NCORES):
        b, g = c // HG, c % HG
        in_maps.append(
            dict(
                xt=xts[b],
                **({"xt8": xt8s[b]} if FP8_QK else {}),
                wq=wqs[g], wk=wks[g], wv=wvs[g],
                cos2=cos2, sin2=sin2, rmat=rmat,
                bq=bqs[g], bk=bks[g], bvb=bvs[g],
                maskt=masks[b],
            )
        )
    return in_maps


def kernel(hidden_states, attention_mask, freqs, Wq, bq, Wk, bk, Wv, bv, **run_kwargs):
    nc = _get_nc()
    in_maps = _prep_inputs(
        hidden_states, attention_mask, freqs, Wq, bq, Wk, bk, Wv, bv
    )
    res = run_bass_kernel_spmd(nc, in_maps, core_ids=list(range(NCORES)), **run_kwargs)
    out = np.empty((B, S, H * DH), np.float32)
    for c in range(NCORES):
        b, g = c // HG, c % HG
        out[b, :, g * OD : (g + 1) * OD] = np.asarray(res.results[c]["out"]).T
    if run_kwargs:
        kernel.last_results = res
    return out

